# revision 2
# baseline (speedup 1.0000x reference)
import sys
sys.path.insert(0, '/opt/trn_rl_repo')
import numpy as np
import ml_dtypes
import concourse.bass as bass
import concourse.bacc as bacc
import concourse.tile as tile
from concourse import mybir
from concourse.bass_utils import run_bass_kernel_spmd

F32 = mybir.dt.float32
BF16 = mybir.dt.bfloat16
AX = mybir.AxisListType
AF = mybir.ActivationFunctionType
BF16NP = ml_dtypes.bfloat16

T, B, CK, CV, H, W = 4, 2, 256, 512, 96, 96
HW = H * W                 # 9216
HHW = HW // 2              # 4608 (3 of 6 row-blocks)
NCORES = 8
NS = HW // NCORES          # 1152 query positions per core
NCH = 3                    # n-chunks per query shard
NW = NS // NCH             # 384
G = T * B                  # 8 frames, one per core
# (si, s, row offset of scale block inside the 50 pooled values)
SCALES = [(0, 1, 49), (1, 2, 45), (2, 3, 36), (3, 6, 0)]
BLK = {1: 9216, 2: 2304, 3: 1024, 6: 256}  # elements summed per pooled value


def _resize_mat(s):
    # 1D bilinear (half-pixel centers, triangle kernel, row-normalized) —
    # matches jax.image.resize(method='bilinear') for upsampling s -> 6.
    out = np.zeros((6, s), np.float64)
    for i in range(6):
        x = (i + 0.5) * s / 6.0 - 0.5
        for p in range(s):
            out[i, p] = max(0.0, 1.0 - abs(x - p))
        out[i] /= out[i].sum()
    return out


_BUILT = {}


def _tree_half(nc, s1p, xt, pl, half):
    """16x16 block sums of one half tile xt [128, 4608] (3 row-blocks of 16
    rows x 96 cols) -> pl[:, half*18 : half*18+18].  Packed-add tree keeps a
    96-wide packed last dim so DVE runs in 2x mode on the big levels."""
    a1 = s1p.tile([128, 2304], BF16, tag="a1")
    x4 = xt[:].rearrange("p (hb hi w) -> p hb hi w", hb=3, hi=16, w=96)
    a14 = a1[:].rearrange("p (hb hi w) -> p hb hi w", hb=3, hi=8, w=96)
    nc.vector.tensor_add(a14[:, :, :, :], x4[:, :, 0:8, :], x4[:, :, 8:16, :])
    a2 = s1p.tile([128, 1152], BF16, tag="a2")
    a24 = a2[:].rearrange("p (hb hi w) -> p hb hi w", hb=3, hi=4, w=96)
    nc.vector.tensor_add(a24[:, :, :, :], a14[:, :, 0:4, :], a14[:, :, 4:8, :])
    a3 = s1p.tile([128, 576], BF16, tag="a3")
    a34 = a3[:].rearrange("p (hb hi w) -> p hb hi w", hb=3, hi=2, w=96)
    nc.vector.tensor_add(a34[:, :, :, :], a24[:, :, 0:2, :], a24[:, :, 2:4, :])
    hs = s1p.tile([128, 288], BF16, tag="hs")
    hs4 = hs[:].rearrange("p (hb w) -> p hb w", hb=3, w=96)
    nc.vector.tensor_add(hs4[:, :, :], a34[:, :, 0:1, :], a34[:, :, 1:2, :])
    nc.vector.reduce_sum(
        pl[:, half * 18:half * 18 + 18],
        hs[:].rearrange("p (hw wi) -> p hw wi", hw=18, wi=16), axis=AX.X)


def _tree_half_pool(nc, p1p, xt, pl, half):
    """Same as _tree_half but entirely on the Pool engine (adds only), to
    relieve DVE during the value stream.  ~3.5x slower per element but Pool
    is otherwise idle through phase 1."""
    a1 = p1p.tile([128, 2304], BF16, tag="pa1")
    x4 = xt[:].rearrange("p (hb hi w) -> p hb hi w", hb=3, hi=16, w=96)
    a14 = a1[:].rearrange("p (hb hi w) -> p hb hi w", hb=3, hi=8, w=96)
    nc.gpsimd.tensor_add(a14[:, :, :, :], x4[:, :, 0:8, :],
                         x4[:, :, 8:16, :])
    a2 = p1p.tile([128, 1152], BF16, tag="pa2")
    a24 = a2[:].rearrange("p (hb hi w) -> p hb hi w", hb=3, hi=4, w=96)
    nc.gpsimd.tensor_add(a24[:, :, :, :], a14[:, :, 0:4, :],
                         a14[:, :, 4:8, :])
    a3 = p1p.tile([128, 576], BF16, tag="pa3")
    a34 = a3[:].rearrange("p (hb hi w) -> p hb hi w", hb=3, hi=2, w=96)
    nc.gpsimd.tensor_add(a34[:, :, :, :], a24[:, :, 0:2, :],
                         a24[:, :, 2:4, :])
    hs = p1p.tile([128, 288], BF16, tag="phs")
    hs4 = hs[:].rearrange("p (hb w) -> p hb w", hb=3, w=96)
    nc.gpsimd.tensor_add(hs4[:, :, :], a34[:, :, 0:1, :], a34[:, :, 1:2, :])
    # w reduce as a packed add tree (gpsimd has no free-axis reduce)
    w1 = p1p.tile([128, 144], BF16, tag="pw1")
    h5 = hs[:].rearrange("p (hw wi) -> p hw wi", hw=18, wi=16)
    w15 = w1[:].rearrange("p (hw wi) -> p hw wi", hw=18, wi=8)
    nc.gpsimd.tensor_add(w15[:, :, :], h5[:, :, 0:8], h5[:, :, 8:16])
    w2 = p1p.tile([128, 72], BF16, tag="pw2")
    w25 = w2[:].rearrange("p (hw wi) -> p hw wi", hw=18, wi=4)
    nc.gpsimd.tensor_add(w25[:, :, :], w15[:, :, 0:4], w15[:, :, 4:8])
    w3 = p1p.tile([128, 36], BF16, tag="pw3")
    w35 = w3[:].rearrange("p (hw wi) -> p hw wi", hw=18, wi=2)
    nc.gpsimd.tensor_add(w35[:, :, :], w25[:, :, 0:2], w25[:, :, 2:4])
    nc.gpsimd.tensor_add(
        pl[:, half * 18:half * 18 + 18],
        w3[:].rearrange("p (hw wi) -> p hw wi", hw=18, wi=2)[:, :, 0:1],
        w3[:].rearrange("p (hw wi) -> p hw wi", hw=18, wi=2)[:, :, 1:2])


def _derived_scales(nc, pl):
    nc.vector.reduce_sum(
        pl[:, 36:45],
        pl[:, 0:36].rearrange("p (a i b j) -> p a b i j",
                              a=3, i=2, b=3, j=2), axis=AX.XY)
    nc.vector.reduce_sum(
        pl[:, 45:49],
        pl[:, 0:36].rearrange("p (a i b j) -> p a b i j",
                              a=2, i=3, b=2, j=3), axis=AX.XY)
    nc.vector.reduce_sum(pl[:, 49:50], pl[:, 0:36], axis=AX.X)


def _build_nc(repeats=1, phase='full'):
    nc = bacc.Bacc("TRN2", target_bir_lowering=False, debug=False,
                   num_devices=NCORES)
    # per-core frame (t,b): all channels, bf16
    mk = nc.dram_tensor("mk", [2, 128, HW], BF16, kind="ExternalInput")
    mv = nc.dram_tensor("mv", [4, 128, HW], BF16, kind="ExternalInput")
    qk = nc.dram_tensor("qk", [B, CK, NS], BF16, kind="ExternalInput")
    # conv weights for this core's frame t, chunks packed along free dim
    wkd = nc.dram_tensor("wk", [128, 2 * CK], BF16, kind="ExternalInput")
    wvd = nc.dram_tensor("wv", [128, 4 * CV], BF16, kind="ExternalInput")
    # biases + upsample mats packed per scale along free dim (rows 0:s^2)
    bkd = nc.dram_tensor("bk", [36, 4 * G * 64], BF16, kind="ExternalInput")
    bvd = nc.dram_tensor("bv", [36, 4 * G * 128], BF16, kind="ExternalInput")
    uad = nc.dram_tensor("ua", [36, 4 * 36], BF16, kind="ExternalInput")
    outm = nc.dram_tensor("outm", [B, CV, NS], BF16, kind="ExternalOutput")

    from contextlib import ExitStack
    with tile.TileContext(nc) as tc:
        with ExitStack() as stack:
            en = stack.enter_context
            en(nc.allow_low_precision(
                reason="output error budget is ~4x the attention magnitude"))
            inpk = en(tc.tile_pool(name="inpk", bufs=3))
            inpv = en(tc.tile_pool(name="inpv", bufs=2))
            s1p = en(tc.tile_pool(name="s1p", bufs=2))
            p1p = en(tc.tile_pool(name="p1p", bufs=1))
            plp = en(tc.tile_pool(name="plp", bufs=6))
            cst = en(tc.tile_pool(name="cst", bufs=1))
            rlp = en(tc.tile_pool(name="rlp", bufs=1))
            bnk = en(tc.tile_pool(name="bnk", bufs=1))
            qkp = en(tc.tile_pool(name="qkp", bufs=1))
            expp = en(tc.tile_pool(name="exp", bufs=7))
            smm = en(tc.tile_pool(name="smm", bufs=4))
            cvs = en(tc.tile_pool(name="cvs", bufs=2))
            omp = en(tc.tile_pool(name="omp", bufs=1))
            psp = en(tc.tile_pool(name="ps", bufs=4, space="PSUM"))
            psc = en(tc.tile_pool(name="psc", bufs=1, space="PSUM"))
            pss = en(tc.tile_pool(name="pss", bufs=1, space="PSUM"))
            pbc = en(tc.tile_pool(name="pbc", bufs=2, space="PSUM"))
            drp = en(tc.tile_pool(name="dram", bufs=1, space="DRAM"))

            ones = cst.tile([128, 128], BF16, tag="ones")
            nc.vector.memset(ones[:], 1.0)
            warm = cst.tile([1, 2], BF16, tag="warm")
            nc.scalar.activation(warm[:], ones[0:1, 0:2], AF.Exp)
            # key-side constants up front on SP
            wk_sb = cst.tile([128, 2 * CK], BF16, tag="wk")
            nc.sync.dma_start(wk_sb[:], wkd[:])
            ua = cst.tile([36, 4 * 36], BF16, tag="ua")
            nc.sync.dma_start(ua[:], uad[:])
            u_sb = [ua[0:s * s, si * 36:(si + 1) * 36]
                    for si, s, off in SCALES]
            # bias constants early (tiny; off the value-stream FIFO)
            rkb = cst.tile([36, 4 * G * 64], BF16, tag="bk")
            nc.sync.dma_start(rkb[:], bkd[:])
            rvb = cst.tile([36, 4 * G * 128], BF16, tag="bv")
            nc.sync.dma_start(rvb[:], bvd[:])

            cbk = drp.tile([50, 64], BF16, tag="cbk")
            cbv = drp.tile([50, 128], BF16, tag="cbv")
            agk = drp.tile([G, 50, 64], BF16, tag="agk")
            agv = drp.tile([G, 50, 128], BF16, tag="agv")

            for _rep in range(repeats):
                # ---- A1: pool + conv, keys of this core's (t,b) frame ------
                kps = psc.tile([50, CK], F32, tag="psc")
                for c in range(2):
                    pl = plp.tile([128, 50], BF16, tag="pl")
                    for hf in range(2):
                        xt = inpk.tile([128, HHW], BF16, tag="xt")
                        nc.sync.dma_start(xt[:], mk[c, :,
                                                    hf * HHW:(hf + 1) * HHW])
                        _tree_half(nc, s1p, xt, pl, hf)
                    _derived_scales(nc, pl)
                    nc.tensor.matmul(kps[:], pl[:],
                                     wk_sb[:, c * CK:(c + 1) * CK],
                                     start=(c == 0), stop=(c == 1))
                stgk = cvs.tile([50, CK], BF16, tag="cvk")
                nc.scalar.copy(stgk[:], kps[:])
                for si, s, off in SCALES:
                    s2 = s * s
                    nc.scalar.dma_start(
                        cbk[off:off + s2, :],
                        stgk[off:off + s2, si * 64:(si + 1) * 64])
                # key gather runs while values are still pooling
                if phase == 'noar':
                    nc.scalar.dma_start(agk[0], cbk[:])
                else:
                    nc.gpsimd.collective_compute(
                        "AllGather", mybir.AluOpType.bypass,
                        replica_groups=[list(range(NCORES))],
                        ins=[cbk.opt()], outs=[agk.opt()])

                # ---- A2: pool + conv, values ------------------------------
                vps = psc.tile([50, CV], F32, tag="psc")
                wv_sb = cst.tile([128, 4 * CV], BF16, tag="wv")
                for c in range(4):
                    pl = plp.tile([128, 50], BF16, tag="pl")
                    for hf in range(2):
                        xt = inpv.tile([128, HHW], BF16, tag="xt")
                        nc.sync.dma_start(xt[:], mv[c, :,
                                                    hf * HHW:(hf + 1) * HHW])
                        _tree_half(nc, s1p, xt, pl, hf)
                        if c == 0 and hf == 0:
                            # value weights: needed from the first vps matmul
                            nc.sync.dma_start(wv_sb[:], wvd[:])
                    _derived_scales(nc, pl)
                    with tc.high_priority():
                        nc.tensor.matmul(vps[:], pl[:],
                                         wv_sb[:, c * CV:(c + 1) * CV],
                                         start=(c == 0), stop=(c == 3))
                with tc.high_priority():
                    stgv = cvs.tile([50, CV], BF16, tag="cvv")
                    nc.vector.tensor_copy(stgv[:], vps[:])
                    for si, s, off in SCALES:
                        s2 = s * s
                        nc.sync.dma_start(
                            cbv[off:off + s2, :],
                            stgv[off:off + s2, si * 128:(si + 1) * 128])
                    if phase == 'noar':
                        nc.scalar.dma_start(agv[0], cbv[:])
                    else:
                        nc.gpsimd.collective_compute(
                            "AllGather", mybir.AluOpType.bypass,
                            replica_groups=[list(range(NCORES))],
                            ins=[cbv.opt()], outs=[agv.opt()])

                # query-key loads (DMA engine is free during the gather)
                qkh = [[qkp.tile([128, NS], BF16, name=f"qkh{b}{h}",
                                 tag=f"qkh{b}{h}") for h in range(2)]
                       for b in range(B)]
                for b in range(B):
                    for h in range(2):
                        nc.sync.dma_start(qkh[b][h][:],
                                          qk[b, 128 * h:128 * (h + 1), :])

                if phase == 'pool':
                    continue

                # ---- bank_k: gated by gather_k, overlaps value gather ------
                stack.enter_context(tc.tile_wait_until(0.046))
                bkh = [[bnk.tile([128, 144], BF16, name=f"bkh{b}{h}",
                                 tag=f"bkh{b}{h}") for h in range(2)]
                       for b in range(B)]
                for si, s, off in SCALES:
                    s2 = s * s
                    rk_t = rlp.tile([s2, G * 64], BF16, tag=f"rk{si}")
                    nc.scalar.dma_start(
                        rk_t[:].rearrange("v (g o) -> v g o", g=G),
                        agk[:, off:off + s2, :].rearrange("g v o -> v g o"))
                    nc.gpsimd.tensor_add(rk_t[:], rk_t[:],
                                         rkb[0:s2, si * 512:(si + 1) * 512])
                    nc.scalar.activation(rk_t[:], rk_t[:], AF.Relu)
                    for b in range(B):
                        ps_bk = psp.tile([64, 144], F32, tag="ps")
                        for t in range(T):
                            g = t * B + b
                            nc.tensor.matmul(
                                ps_bk[:, t * 36:(t + 1) * 36],
                                rk_t[:, g * 64:(g + 1) * 64],
                                u_sb[si], start=True, stop=True)
                        h, r = si // 2, (si % 2) * 64
                        nc.scalar.copy(bkh[b][h][r:r + 64, :], ps_bk[:])

                # ---- attention softmax: overlaps the value gather ----------
                stack.enter_context(tc.tile_wait_until(0.052))
                # memory dim M=144 split 108 (t0-2) + 36 (t3: served directly
                # from the stg36 staging tile, no bank assembly DMA)
                exs = []
                for b in range(B):
                    for ch in range(NCH):
                        cs = slice(ch * NW, (ch + 1) * NW)
                        aps0 = pbc.tile([108, NW], F32, tag="psa")
                        aps1 = pss.tile([36, NW], F32, tag="pssm")
                        for h in range(2):
                            nc.tensor.matmul(aps0[:], bkh[b][h][:, 0:108],
                                             qkh[b][h][:, cs],
                                             start=(h == 0), stop=(h == 1))
                        for h in range(2):
                            nc.tensor.matmul(aps1[:], bkh[b][h][:, 108:144],
                                             qkh[b][h][:, cs],
                                             start=(h == 0), stop=(h == 1))
                        ex0 = expp.tile([108, NW], BF16, name="ex0",
                                        tag="ex0")
                        ex1 = expp.tile([36, NW], BF16, name="ex1", tag="ex1")
                        nc.scalar.activation(ex0[:], aps0[:], AF.Exp,
                                             scale=1.0 / 16.0)
                        nc.scalar.activation(ex1[:], aps1[:], AF.Exp,
                                             scale=1.0 / 16.0)
                        sums = pss.tile([1, NW], F32, tag="pssm")
                        nc.tensor.matmul(sums[:], ones[0:108, 0:1], ex0[:],
                                         start=True, stop=False)
                        nc.tensor.matmul(sums[:], ones[0:36, 0:1], ex1[:],
                                         start=False, stop=True)
                        rcp = smm.tile([1, NW], BF16, name="rcp", tag="rcp")
                        nc.vector.reciprocal(rcp[:], sums[:])
                        bc = pbc.tile([108, NW], F32, tag="psa")
                        nc.tensor.matmul(bc[:], ones[0:1, 0:108], rcp[:],
                                         start=True, stop=True)
                        # normalize probs in place: mem matmul output is final
                        nc.vector.tensor_mul(ex0[:], ex0[:], bc[:])
                        nc.vector.tensor_mul(ex1[:], ex1[:], bc[0:36, :])
                        exs.append((ex0, ex1))

                # ---- bank_v: gated by gather_v -----------------------------
                stack.enter_context(tc.tile_wait_until(0.062))
                rv_sb = {}
                for si, s, off in SCALES:
                    s2 = s * s
                    rv_t = rlp.tile([s2, G * 128], BF16, tag=f"rv{si}")
                    nc.sync.dma_start(
                        rv_t[:].rearrange("v (g o) -> v g o", g=G),
                        agv[:, off:off + s2, :].rearrange("g v o -> v g o"))
                    nc.vector.tensor_add(rv_t[:], rv_t[:],
                                         rvb[0:s2, si * 1024:(si + 1) * 1024])
                    nc.vector.tensor_scalar_max(rv_t[:], rv_t[:], 0.0)
                    rv_sb[si] = rv_t
                # bv0 [108, 512] holds t0-2; t3 is read straight out of the
                # stg36 staging tile (base partition 0, no assembly DMA)
                bv0 = [bnk.tile([108, 512], BF16, name=f"bv0{b}",
                                tag=f"bv0{b}") for b in range(B)]
                stg36s = [smm.tile([36, 2048], BF16, name=f"st36{b}",
                                   tag=f"st36{b}") for b in range(B)]
                for b in range(B):
                    stg36 = stg36s[b]
                    for t in range(T):
                        g = t * B + b
                        ps_bv = psp.tile([36, 512], F32, tag="ps")
                        for si, s, off in SCALES:
                            nc.tensor.matmul(
                                ps_bv[:, si * 128:(si + 1) * 128],
                                u_sb[si],
                                rv_sb[si][:, g * 128:(g + 1) * 128],
                                start=True, stop=True)
                        ceng = (nc.scalar, nc.vector, nc.scalar,
                                nc.vector)[t]
                        dst = stg36[:, t * 512:(t + 1) * 512]
                        if ceng is nc.scalar:
                            ceng.copy(dst, ps_bv[:])
                        else:
                            ceng.tensor_copy(dst, ps_bv[:])
                        if t < 3:
                            nc.sync.dma_start(bv0[b][t * 36:(t + 1) * 36, :],
                                              dst)

                # ---- weighted values + output ------------------------------
                stack.enter_context(tc.tile_wait_until(0.066))
                for b in range(B):
                    stg36 = stg36s[b]
                    for vc in range(4):
                        vs = slice(vc * 128, (vc + 1) * 128)
                        om = omp.tile([128, NS], BF16, name=f"om{b}{vc}",
                                      tag=f"om{b}{vc}")
                        for ch in range(NCH):
                            cs = slice(ch * NW, (ch + 1) * NW)
                            ex0, ex1 = exs[b * NCH + ch]
                            mps = psp.tile([128, NW], F32, tag="ps")
                            nc.tensor.matmul(mps[:], bv0[b][:, vs], ex0[:],
                                             start=True, stop=False)
                            nc.tensor.matmul(
                                mps[:],
                                stg36[0:36, 3 * 512 + vc * 128:
                                      3 * 512 + (vc + 1) * 128],
                                ex1[:], start=False, stop=True)
                            eng = (nc.vector,
                                   nc.scalar)[(ch + vc) % 2]
                            if eng is nc.scalar:
                                eng.copy(om[:, cs], mps[:])
                            else:
                                eng.tensor_copy(om[:, cs], mps[:])
                        nc.sync.dma_start(
                            outm[b, vc * 128:(vc + 1) * 128, :], om[:])

    nc.compile()
    return nc


def _get_nc(repeats=1, phase='full'):
    key = (repeats, phase)
    if key not in _BUILT:
        _BUILT[key] = _build_nc(repeats, phase)
    return _BUILT[key]


def _host_prep(memory_keys, memory_values, query_key, query_value,
               key_w, key_b, val_w, val_b):
    mk = np.asarray(memory_keys, np.float32).reshape(T, B, CK, HW)
    mv = np.asarray(memory_values, np.float32).reshape(T, B, CV, HW)
    qk = np.asarray(query_key, np.float32).reshape(B, CK, HW)
    kw = np.asarray(key_w, np.float32).copy()
    vw = np.asarray(val_w, np.float32).copy()
    kb = np.asarray(key_b, np.float32)
    vb = np.asarray(val_b, np.float32)
    for si, s, off in SCALES:
        kw[:, si] /= BLK[s]
        vw[:, si] /= BLK[s]

    # biases packed per scale along free dim: [36, 4*G*64] / [36, 4*G*128]
    bk_host = np.zeros((36, 4 * G * 64), np.float32)
    bv_host = np.zeros((36, 4 * G * 128), np.float32)
    for si, s, off in SCALES:
        s2 = s * s
        for t in range(T):
            for b in range(B):
                g = t * B + b
                bk_host[0:s2, si * 512 + g * 64:si * 512 + (g + 1) * 64] = \
                    kb[t, si][None, :]
                bv_host[0:s2,
                        si * 1024 + g * 128:si * 1024 + (g + 1) * 128] = \
                    vb[t, si][None, :]
    bk_host = bk_host.astype(BF16NP)
    bv_host = bv_host.astype(BF16NP)
    ua_host = np.zeros((36, 4 * 36), np.float32)
    for si, s, off in SCALES:
        R = _resize_mat(s)
        U = np.einsum('ip,jq->pqij', R, R).reshape(s * s, 36)
        ua_host[0:s * s, si * 36:(si + 1) * 36] = U
    ua_host = ua_host.astype(BF16NP)

    in_maps = []
    for k in range(NCORES):
        t, b = k // B, k % B
        wk_full = kw[t].transpose(2, 0, 1).reshape(CK, CK)    # [c, si*64+o]
        wv_full = vw[t].transpose(2, 0, 1).reshape(CV, CV)    # [c, si*128+o]
        # chunk along cin, pack chunks along free dim: [128, 2*CK]/[128, 4*CV]
        wk_p = np.concatenate([wk_full[i * 128:(i + 1) * 128]
                               for i in range(2)], axis=1)
        wv_p = np.concatenate([wv_full[i * 128:(i + 1) * 128]
                               for i in range(4)], axis=1)
        m = {
            "mk": np.ascontiguousarray(
                mk[t, b].reshape(2, 128, HW).astype(BF16NP)),
            "mv": np.ascontiguousarray(
                mv[t, b].reshape(4, 128, HW).astype(BF16NP)),
            "qk": np.ascontiguousarray(
                qk[:, :, k * NS:(k + 1) * NS].astype(BF16NP)),
            "wk": np.ascontiguousarray(wk_p.astype(BF16NP)),
            "wv": np.ascontiguousarray(wv_p.astype(BF16NP)),
            "bk": bk_host, "bv": bv_host, "ua": ua_host,
        }
        in_maps.append(m)
    return in_maps


def kernel(**inputs):
    nc = _get_nc()
    in_maps = _host_prep(**inputs)
    res = run_bass_kernel_spmd(nc, in_maps, core_ids=list(range(NCORES)),
                               trace=False)
    shards = [res.results[i]["outm"].astype(np.float32)
              for i in range(NCORES)]
    mem = np.concatenate(shards, axis=2)              # [B, CV, HW]
    qv = np.asarray(inputs['query_value'], np.float32).reshape(B, CV, HW)
    full = np.concatenate([qv, mem], axis=1).reshape(B, 2 * CV, H, W)
    return full
